# revision 20
# baseline (speedup 1.0000x reference)
"""3-layer 'bidirectional' (all-forward) GRU encoder on 8 trn2 cores.

Strategy: data-parallel over batch (64 -> 8 per core). All tensor layout
transposes happen host-side in numpy (free); on-chip everything is H-major
(gate/hidden index on partitions, time*batch on the free dim).

Per core:
  phase A: load x.T (300, L*8) + packed weights.
  phase gi(l): gi = x_l @ W_ih_l.T for ALL timesteps as one big tiled matmul
     (bias folded in during the PSUM->SBUF copy; for r,z gates b_hh is
     pre-added host-side; b_hh_n is applied via a K=76 ones-row matmul trick).
  phase scan(l): 512 sequential steps; per step only the 6 small gh matmuls
     (3 gates x 2 dirs, K=75/76, M=75, N=8) + gate elementwise.
     Layer l+1's gi phase reads layer l's SBUF-resident H-major history
     directly as the matmul moving operand (no transposes anywhere).
  output: contiguous H-major DMA out; host transposes back.
"""

import os

import numpy as np

L = int(os.environ.get("BASSGRU_L", "512"))
B, IN, H = 64, 300, 75
NCORES = 8
BS = B // NCORES  # batch shard per core = 8
PW = 2 * BS  # pair width (dir0 | dir1) = 16
GB = 6 * BS  # gi block per timestep = 48
NT = L * BS  # moving-dim total for big matmuls
NCHUNK = min(512, NT)  # matmul N-chunk (one PSUM bank of fp32)

_F32 = np.float32


# --------------------------------------------------------------------------
# host-side packing (pure numpy; validated against reference by test harness)
# --------------------------------------------------------------------------

def pack_weights(w_ih0, w_hh0, b_ih0, b_hh0, w_ih12, w_hh12, b_ih12, b_hh12):
    """Pack weights into the on-chip layouts. Column block (g*2+d)*75 for
    gate g in (r,z,n), direction d."""
    w_ih0 = np.asarray(w_ih0, _F32)
    w_hh0 = np.asarray(w_hh0, _F32)
    b_ih0 = np.asarray(b_ih0, _F32)
    b_hh0 = np.asarray(b_hh0, _F32)
    w_ih12 = np.asarray(w_ih12, _F32)
    w_hh12 = np.asarray(w_hh12, _F32)
    b_ih12 = np.asarray(b_ih12, _F32)
    b_hh12 = np.asarray(b_hh12, _F32)

    # WI0T (300, 450)
    blocks = [w_ih0[d, g * H:(g + 1) * H, :] for g in range(3) for d in range(2)]
    wi0 = np.ascontiguousarray(np.concatenate(blocks, axis=0).T)

    # WI12T (2, 150, 450)
    wi12 = np.zeros((2, 2 * H, 6 * H), _F32)
    for l in range(2):
        blocks = [w_ih12[l, d, g * H:(g + 1) * H, :] for g in range(3) for d in range(2)]
        wi12[l] = np.concatenate(blocks, axis=0).T

    # WHH (76, 3*450): row 75 carries b_hh_n for the n-gate blocks
    whh = np.zeros((H + 1, 3 * 6 * H), _F32)
    for l in range(3):
        whh_l = w_hh0 if l == 0 else w_hh12[l - 1]
        bhh_l = b_hh0 if l == 0 else b_hh12[l - 1]
        for g in range(3):
            for d in range(2):
                col = l * 6 * H + (g * 2 + d) * H
                whh[0:H, col:col + H] = whh_l[d, g * H:(g + 1) * H, :].T
                if g == 2:  # n gate: bias rides the ones-row
                    whh[H, col:col + H] = bhh_l[d, g * H:(g + 1) * H]

    # BIAS (75, 24): col l*8 + (g*2+d); r,z get b_ih + b_hh, n gets b_ih only
    bias = np.zeros((H, 24), _F32)
    for l in range(3):
        bih_l = b_ih0 if l == 0 else b_ih12[l - 1]
        bhh_l = b_hh0 if l == 0 else b_hh12[l - 1]
        for g in range(3):
            for d in range(2):
                v = bih_l[d, g * H:(g + 1) * H]
                if g < 2:
                    v = v + bhh_l[d, g * H:(g + 1) * H]
                bias[:, l * 8 + g * 2 + d] = v
    return wi0, wi12, whh, bias


def pack_core_inputs(input_, h0, core):
    """Per-core x.T (300, L*BS) and packed h0 (75, 48)."""
    input_ = np.asarray(input_, _F32)
    h0 = np.asarray(h0, _F32)
    sl = slice(core * BS, (core + 1) * BS)
    xT = np.ascontiguousarray(input_[:L, sl, :].transpose(2, 0, 1).reshape(IN, L * BS))
    h0p = np.zeros((H, 3 * PW), _F32)
    for l in range(3):
        for d in range(2):
            # (BS, H) -> (H, BS)
            h0p[:, l * PW + d * BS:l * PW + (d + 1) * BS] = h0[l * 2 + d, sl, :].T
    return xT, h0p


def assemble_output(core_outs):
    """core_outs[i]: (3, 75, L, PW) H-major -> full (L, B, 450)."""
    out = np.empty((L, B, 6 * H), _F32)
    for s, a in enumerate(core_outs):
        # (l, j, t, d, b) -> (t, b, l, d, j)
        out[:, s * BS:(s + 1) * BS, :] = (
            a.reshape(3, H, L, 2, BS).transpose(2, 4, 0, 3, 1).reshape(L, BS, 6 * H)
        )
    return out


def numpy_sim(**inputs):
    """Numpy simulation of the exact on-chip compute/layout (for validating
    the packing math without compiling)."""
    wi0, wi12, whh, bias = pack_weights(
        inputs["w_ih0"], inputs["w_hh0"], inputs["b_ih0"], inputs["b_hh0"],
        inputs["w_ih12"], inputs["w_hh12"], inputs["b_ih12"], inputs["b_hh12"])

    def sigmoid(x):
        return 1.0 / (1.0 + np.exp(-x))

    core_outs = []
    for s in range(NCORES):
        xT, h0p = pack_core_inputs(inputs["input"], inputs["h0"], s)
        out = np.zeros((3, H, L, PW), _F32)
        gi_prev = None  # H-major history of previous layer (76, (L+1)*PW)
        for l in range(3):
            # gi phase: (450, NT) H-major
            if l == 0:
                gi = wi0.T @ xT  # (450, NT)
            else:
                x_l = gi_prev  # (150, NT): rows d*75+j, cols t*BS+b
                gi = wi12[l - 1].T @ x_l
            gi = gi + bias[:, l * 8:l * 8 + 6].T.reshape(6 * H, 1)
            # scan
            hs = np.ones((H + 1, (L + 1) * PW), _F32)
            hs[0:H, 0:PW] = h0p[:, l * PW:(l + 1) * PW]
            for t in range(L):
                hp = hs[:, t * PW:(t + 1) * PW]  # (76, 16) with ones row
                ps = np.zeros((H, GB), _F32)
                for g in range(3):
                    for d in range(2):
                        col = l * 6 * H + (g * 2 + d) * H
                        kk = H + 1 if g == 2 else H
                        ps[:, (g * 2 + d) * BS:(g * 2 + d + 1) * BS] = (
                            whh[0:kk, col:col + H].T @ hp[0:kk, d * BS:(d + 1) * BS])
                gi_t = np.empty((H, GB), _F32)
                for g in range(3):
                    for d in range(2):
                        gi_t[:, (g * 2 + d) * BS:(g * 2 + d + 1) * BS] = gi[
                            (g * 2 + d) * H:(g * 2 + d + 1) * H, t * BS:(t + 1) * BS]
                rz = sigmoid(ps[:, 0:2 * PW] + gi_t[:, 0:2 * PW])
                t1 = rz[:, 0:PW] * ps[:, 2 * PW:3 * PW]
                npre = t1 + gi_t[:, 2 * PW:3 * PW]
                n = np.tanh(npre)
                dd = hp[0:H] - n
                t2 = rz[:, PW:2 * PW] * dd
                hs[0:H, (t + 1) * PW:(t + 2) * PW] = n + t2
            out[l] = hs[0:H, PW:].reshape(H, L, PW)
            # next layer's input: (150, NT): rows d*75+j
            nxt = np.empty((2 * H, NT), _F32)
            for d in range(2):
                nxt[d * H:(d + 1) * H] = (
                    hs[0:H, PW:].reshape(H, L, 2, BS)[:, :, d, :].reshape(H, NT))
            gi_prev = nxt
        core_outs.append(out)
    return assemble_output(core_outs)


# --------------------------------------------------------------------------
# bass program
# --------------------------------------------------------------------------

_CACHED = {}


def _build_program():
    from contextlib import ExitStack

    import concourse.bacc as bacc
    import concourse.mybir as mybir
    import concourse.tile as tile

    f32 = mybir.dt.float32
    AF = mybir.ActivationFunctionType

    nc = bacc.Bacc("TRN2", target_bir_lowering=False, debug=False,
                   num_devices=NCORES)

    xT_d = nc.dram_tensor("xT", [IN, NT], f32, kind="ExternalInput").ap()
    wi0_d = nc.dram_tensor("wi0", [IN, 6 * H], f32, kind="ExternalInput").ap()
    wi12_d = nc.dram_tensor("wi12", [2, 2 * H, 6 * H], f32, kind="ExternalInput").ap()
    whh_d = nc.dram_tensor("whh", [H + 1, 18 * H], f32, kind="ExternalInput").ap()
    bias_d = nc.dram_tensor("bias", [H, 24], f32, kind="ExternalInput").ap()
    h0p_d = nc.dram_tensor("h0p", [H, 3 * PW], f32, kind="ExternalInput").ap()
    ones_d = nc.dram_tensor("ones", [1, (L + 1) * PW], f32, kind="ExternalInput").ap()
    chain_d = nc.dram_tensor("chain", [1, 8], f32, kind="ExternalInput").ap()
    chout_d = nc.dram_tensor("chain_out", [1, 8], f32, kind="ExternalOutput").ap()
    out_d = nc.dram_tensor("out", [3, H, L, PW], f32, kind="ExternalOutput").ap()

    nch = NT // NCHUNK  # n-chunks for big matmuls
    tch = NCHUNK // BS  # timesteps per n-chunk

    with tile.TileContext(nc) as tc, ExitStack() as ctx:
        wpool = ctx.enter_context(tc.tile_pool(name="weights", bufs=1))
        gi_pool = ctx.enter_context(tc.tile_pool(name="gi", bufs=2))
        bf16 = mybir.dt.bfloat16
        ps_big = ctx.enter_context(tc.tile_pool(name="psbig", bufs=2, space="PSUM"))
        ps_scan = ctx.enter_context(tc.tile_pool(name="psscan", bufs=6, space="PSUM"))
        sm = ctx.enter_context(tc.tile_pool(name="smalls", bufs=4))

        # ---- load weights
        whh_t = wpool.tile([H + 1, 18 * H], f32, tag="whh")
        nc.sync.dma_start(whh_t[:], whh_d[:])
        bias_t = wpool.tile([H, 24], f32, tag="bias")
        nc.sync.dma_start(bias_t[:], bias_d[:])
        # wi12 split into the two K=75 chunks (dir of source layer);
        # layers side by side on the free dim
        wi12_t = [wpool.tile([H, 2 * 6 * H], f32, tag=f"wi12_{k}", name=f"wi12_{k}")
                  for k in range(2)]
        for l in range(2):
            for k in range(2):
                nc.sync.dma_start(wi12_t[k][:, l * 6 * H:(l + 1) * 6 * H],
                                  wi12_d[l, k * H:(k + 1) * H, :])

        def gi_phase(l, gi_store, hs_prev, xt_tiles, wi0_sb=None):
            """gi_store (75, L*GB) <- W_ih_l.T @ x_l for all t, + bias."""
            for mb in range(6):
                bcol = l * 8 + mb
                for c in range(nch):
                    ps = ps_big.tile([H, NCHUNK], f32, tag="gips")
                    if l == 0:
                        for k in range(3):
                            nc.tensor.matmul(
                                ps[:],
                                wi0_sb[k][:, mb * H:(mb + 1) * H],
                                xt_tiles[k][:, c * NCHUNK:(c + 1) * NCHUNK],
                                start=(k == 0), stop=(k == 2))
                    else:
                        hsv = hs_prev[0:H].rearrange("p (t w) -> p t w", w=PW)
                        for k in range(2):
                            nc.tensor.matmul(
                                ps[:],
                                wi12_t[k][:, (l - 1) * 6 * H + mb * H:
                                          (l - 1) * 6 * H + (mb + 1) * H],
                                hsv[:, 1 + c * tch: 1 + (c + 1) * tch,
                                    k * BS:(k + 1) * BS],
                                start=(k == 0), stop=(k == 1))
                    # copy + bias into strided gi_store
                    dst = gi_store[:].rearrange("p (t g) -> p t g", g=GB)[
                        :, c * tch:(c + 1) * tch, mb * BS:(mb + 1) * BS]
                    psv = ps[:].rearrange("p (a b) -> p a b", b=BS)
                    nc.scalar.activation(dst, psv, AF.Identity,
                                         bias=bias_t[:, bcol:bcol + 1], scale=1.0)

        def scan_phase(l, gi_store, hs):
            nc.sync.dma_start(hs[H:H + 1, :], ones_d[:])
            nc.sync.dma_start(hs[0:H, 0:PW], h0p_d[:, l * PW:(l + 1) * PW])
            for t in range(L):
                ps = ps_scan.tile([H, GB], f32, tag=f"sps{l}", bufs=2)
                for g in range(3):
                    for d in range(2):
                        mb = g * 2 + d
                        col = l * 6 * H + mb * H
                        kk = H + 1 if g == 2 else H
                        nc.tensor.matmul(
                            ps[:, mb * BS:(mb + 1) * BS],
                            whh_t[0:kk, col:col + H],
                            hs[0:kk, t * PW + d * BS:t * PW + (d + 1) * BS],
                            start=True, stop=True)
                gi_t = gi_store[:, t * GB:(t + 1) * GB]
                nc.vector.tensor_add(ps[:, 0:2 * PW], ps[:, 0:2 * PW],
                                     gi_t[:, 0:2 * PW])
                rz = sm.tile([H, 2 * PW], f32, tag=f"rz{l}")
                nc.scalar.activation(rz[:], ps[:, 0:2 * PW], AF.Sigmoid)
                t1 = sm.tile([H, PW], f32, tag=f"t1{l}")
                nc.vector.tensor_mul(t1[:], rz[:, 0:PW], ps[:, 2 * PW:3 * PW])
                npre = sm.tile([H, PW], f32, tag=f"npre{l}")
                nc.vector.tensor_add(npre[:], t1[:], gi_t[:, 2 * PW:3 * PW])
                nsb = sm.tile([H, PW], f32, tag=f"n{l}")
                nc.scalar.activation(nsb[:], npre[:], AF.Tanh)
                dsb = sm.tile([H, PW], f32, tag=f"d{l}")
                nc.gpsimd.tensor_sub(dsb[:], hs[0:H, t * PW:(t + 1) * PW], nsb[:])
                t2 = sm.tile([H, PW], f32, tag=f"t2{l}")
                nc.gpsimd.tensor_mul(t2[:], rz[:, PW:2 * PW], dsb[:])
                nc.vector.tensor_add(hs[0:H, (t + 1) * PW:(t + 2) * PW],
                                     nsb[:], t2[:])
            nc.sync.dma_start(out_d[l].rearrange("p t w -> p (t w)"),
                              hs[0:H, PW:(L + 1) * PW])

        # chain passthrough (lets the bench serialize repeated executions)
        ch = wpool.tile([1, 8], f32, tag="chain", name="ch")
        nc.sync.dma_start(ch[:], chain_d[:])
        nc.sync.dma_start(chout_d[:], ch[:])

        # ---- layer 0
        gi_store = gi_pool.tile([H, L * GB], bf16, tag="gi")
        with tc.tile_pool(name="xt", bufs=1) as xt_pool:
            xt_tiles = [xt_pool.tile([100, NT], f32, tag=f"xt{k}", name=f"xt{k}") for k in range(3)]
            wi0_sb = [xt_pool.tile([100, 6 * H], f32, tag=f"wi0_{k}", name=f"wi0s_{k}") for k in range(3)]
            for k in range(3):
                nc.sync.dma_start(xt_tiles[k][:], xT_d[k * 100:(k + 1) * 100, :])
                nc.sync.dma_start(wi0_sb[k][:], wi0_d[k * 100:(k + 1) * 100, :])
            gi_phase(0, gi_store, None, xt_tiles, wi0_sb)
        hs_pool = ctx.enter_context(tc.tile_pool(name="hs", bufs=2))
        hs0 = hs_pool.tile([H + 1, (L + 1) * PW], f32, tag="hs", name="hs0")
        scan_phase(0, gi_store, hs0)

        # ---- layer 1
        if os.environ.get("BASSGRU_LAYERS", "3") == "1":
            return nc
        gi_store1 = gi_pool.tile([H, L * GB], bf16, tag="gi")
        gi_phase(1, gi_store1, hs0, None)
        hs1 = hs_pool.tile([H + 1, (L + 1) * PW], f32, tag="hs", name="hs1")
        scan_phase(1, gi_store1, hs1)

        # ---- layer 2
        if os.environ.get("BASSGRU_LAYERS", "3") == "2":
            return nc
        gi_store2 = gi_pool.tile([H, L * GB], bf16, tag="gi")
        gi_phase(2, gi_store2, hs1, None)
        hs2 = hs_pool.tile([H + 1, (L + 1) * PW], f32, tag="hs", name="hs2")
        scan_phase(2, gi_store2, hs2)

    nc.compile()
    return nc


def kernel(**inputs):
    import concourse.bass_utils as bass_utils

    if "nc" not in _CACHED:
        _CACHED["nc"] = _build_program()
    nc = _CACHED["nc"]

    wi0, wi12, whh, bias = pack_weights(
        inputs["w_ih0"], inputs["w_hh0"], inputs["b_ih0"], inputs["b_hh0"],
        inputs["w_ih12"], inputs["w_hh12"], inputs["b_ih12"], inputs["b_hh12"])
    in_maps = []
    for s in range(NCORES):
        xT, h0p = pack_core_inputs(inputs["input"], inputs["h0"], s)
        in_maps.append({"xT": xT, "wi0": wi0, "wi12": wi12, "whh": whh,
                        "bias": bias, "h0p": h0p,
                        "ones": np.ones((1, (L + 1) * PW), _F32),
                        "chain": np.zeros((1, 8), _F32)})

    trace = os.environ.get("BASSGRU_TRACE", "0") == "1"
    res = bass_utils.run_bass_kernel_spmd(nc, in_maps, core_ids=list(range(NCORES)),
                                          trace=trace)
    _CACHED["last_results"] = res
    return assemble_output([r["out"] for r in res.results])
